# revision 7
# baseline (speedup 1.0000x reference)
"""Weighted 2D cross-entropy (BCE-over-classes) loss on 8 Trainium2 cores.

Math (matches the reference):
  t in [0,19); pos = t>0, neg = t==0 (all pixels are pos or neg; mask == 1)
  S(i) = sum_c bce(i,c) = -[ B(i) + A(i) ]
     A(i) = sum_c log(1-p_c(i))
     B(i) = log(p_t(i)) - log(1-p_t(i))
  loss = ( (NEG/TOT)*S_pos_sum + (POS/TOT)*S_neg_sum ) / (TOT*C)

Per-core (core k <- batch element k, pure data parallel):
  - per class-tile: ACT pass L_c = Ln(1-p_c) in bf16 with accum_out
    giving the U_all = sum A column sums for free
  - DVE: eq mask (T==c) and masked_c = eq*L_c
  - PE identity-matmuls accumulate A = sum_c L_c and L_sel = sum_c masked_c
    into PSUM (f32) -- the gather of log(1-p) at the target class.
  - per-tile tail: B = Ln(exp(-L_sel) - 1) = log(p_t) - log(1-p_t) in two
    ACT ops (accum_out on the Ln gives sum B free); two DVE STTs give the
    pos-masked sums of A and B.
  - the last pixel tile is processed in two 512-col halves so the final
    dependency chain after the last DMA byte is short.
  - the whole [128, STAT_COLS] stats tile is DMA'd out; the host does all
    folding (partition+column sums and the cross-core "all-reduce") in f64.
"""

from contextlib import ExitStack

import numpy as np

import concourse.bass as bass
import concourse.mybir as mybir
import concourse.tile as tile
from concourse import bacc
from concourse.bass_utils import run_bass_kernel_spmd

# problem shape (hardcoded per harness contract)
N, C, H, W = 8, 19, 512, 1024
PIX = H * W          # 524288 pixels per core
P = 128              # partitions
FCOLS = PIX // P     # 4096 free columns when pixels laid out [128, 4096]
FT = 1024            # pixel-tile free width
NTILES = FCOLS // FT # 4 pixel tiles per core
HALF = FT // 2       # last tile processed as two 512-col halves
N_CORES = 8

DT = mybir.dt

# stats buffer column layout (all f32, each column written exactly once)
COL_UALL = 0
N_UALL = C * (NTILES - 1) + 2 * C       # 57 full tiles + 38 half tiles = 95
COL_B = COL_UALL + N_UALL               # 95: sum B per tail
N_TAILS = (NTILES - 1) + 2              # 3 full tails + 2 half tails = 5
COL_POSB = COL_B + N_TAILS              # 100: sum pos*B per tail
COL_POSA = COL_POSB + N_TAILS           # 105: sum pos*A per tail
COL_CNT = COL_POSA + N_TAILS            # 110: pos count per target chunk
STAT_COLS = COL_CNT + NTILES            # 114
NSTAT = STAT_COLS  # legacy alias


def build_kernel() -> bass.Bass:
    # Bacc (not raw Bass): its compile() pipeline runs
    # generate_event_semaphores, which splits multi-sem waits to satisfy the
    # 1-wait-per-instruction TRN2 sync structs -- raw Bass modules with
    # Tile-emitted multi-waits fail walrus codegen.
    nc = bacc.Bacc("TRN2")

    predict = nc.declare_dram_parameter("predict", [C, PIX], DT.float32, isOutput=False)
    target = nc.declare_dram_parameter("target", [P, FCOLS], DT.int32, isOutput=False)
    idn = nc.declare_dram_parameter("idn", [P, P], DT.bfloat16, isOutput=False)
    out = nc.declare_dram_parameter("out", [P, STAT_COLS], DT.float32, isOutput=True)

    pred_r = predict.rearrange("c (p f) -> c p f", p=P)  # [19, 128, 4096]

    with tile.TileContext(nc) as tc, ExitStack() as ctx:
        const = ctx.enter_context(tc.tile_pool(name="const", bufs=1))
        p_pool = ctx.enter_context(tc.tile_pool(name="p", bufs=8))
        lm_pool = ctx.enter_context(tc.tile_pool(name="lm", bufs=21))
        pix_pool = ctx.enter_context(tc.tile_pool(name="pix", bufs=2))
        scr_pool = ctx.enter_context(tc.tile_pool(name="scr", bufs=2))
        eq_pool = ctx.enter_context(tc.tile_pool(name="eq", bufs=4))
        psum_pool = ctx.enter_context(tc.tile_pool(name="ps", bufs=2, space="PSUM"))

        # constants + target go through the gpsimd queue so the sync queue's
        # first instruction is the first predict load (DMA starts at t~0)
        idn_sb = const.tile([P, P], DT.bfloat16, tag="idn")
        nc.gpsimd.dma_start(out=idn_sb[:], in_=idn[:])

        t_i32 = const.tile([P, FCOLS], DT.int32, tag="ti")
        for t in range(NTILES):
            sl = slice(t * FT, (t + 1) * FT)
            nc.gpsimd.dma_start(out=t_i32[:, sl], in_=target[:, sl])

        t_bf = const.tile([P, FCOLS], DT.bfloat16, tag="tb")
        stats = const.tile([P, STAT_COLS], DT.float32, tag="stats")
        cnt_scr = const.tile([P, FT], DT.bfloat16, tag="cntscr")
        neg1 = const.tile([P, 1], DT.float32, tag="neg1")
        nc.gpsimd.memset(neg1[:], -1.0)

        def target_chunk_prep(t):
            # int->bf16 conversion + pos count for chunk t; emitted just
            # before tile t's class loop so the DVE queue head never blocks
            # on a not-yet-arrived target chunk
            sl = slice(t * FT, (t + 1) * FT)
            nc.vector.tensor_copy(out=t_bf[:, sl], in_=t_i32[:, sl])
            nc.vector.tensor_scalar(
                out=cnt_scr[:],
                in0=t_bf[:, sl],
                scalar1=0.5,
                scalar2=None,
                op0=mybir.AluOpType.is_gt,
                op1=mybir.AluOpType.add,
                accum_out=stats[:, COL_CNT + t : COL_CNT + t + 1],
            )

        ucol = [COL_UALL]

        def class_pass(p_src, t_sl, lm_w, acc_ps, acc_off, c):
            """One class over lm_w pixel columns: Ln, eq, mask, PE accumulate."""
            lm_full = lm_pool.tile([P, 2 * FT], DT.bfloat16, tag="lm", name="lm")
            lm = lm_full[:, : 2 * lm_w]
            nc.scalar.activation(
                out=lm[:, :lm_w],
                in_=p_src,
                func=mybir.ActivationFunctionType.Ln,
                bias=1.0,
                scale=-1.0,
                accum_out=stats[:, ucol[0] : ucol[0] + 1],
            )
            ucol[0] += 1
            eq_full = eq_pool.tile([P, FT], DT.bfloat16, tag="eq", name="eq")
            eq = eq_full[:, :lm_w]
            nc.vector.tensor_scalar(
                out=eq[:],
                in0=t_sl,
                scalar1=float(c),
                scalar2=None,
                op0=mybir.AluOpType.is_equal,
            )
            nc.vector.tensor_mul(out=lm[:, lm_w:], in0=eq[:], in1=lm[:, :lm_w])
            for s in range(2 * lm_w // 512):
                ssl = slice(s * 512, (s + 1) * 512)
                asl = slice(acc_off + s * 512, acc_off + (s + 1) * 512)
                nc.tensor.matmul(
                    acc_ps[:, asl],
                    lhsT=idn_sb[:],
                    rhs=lm[:, ssl],
                    start=(c == 0),
                    stop=(c == C - 1),
                )

        def emit_tail(t_sl, a_ps, lsel_ps, k, width):
            """Tail over one pixel range: pos*A, B = Ln(exp(-lsel)-1), pos*B."""
            scr = scr_pool.tile([P, FT], DT.float32, tag="scr")
            nc.vector.scalar_tensor_tensor(
                out=scr[:, :width],
                in0=t_sl,
                scalar=0.5,
                in1=a_ps,
                op0=mybir.AluOpType.is_gt,
                op1=mybir.AluOpType.mult,
                accum_out=stats[:, COL_POSA + k : COL_POSA + k + 1],
            )
            expl_full = pix_pool.tile([P, FT], DT.float32, tag="expl", name="expl")
            expl = expl_full[:, :width]
            nc.scalar.activation(
                out=expl[:],
                in_=lsel_ps,
                func=mybir.ActivationFunctionType.Exp,
                scale=-1.0,
            )
            bq_full = pix_pool.tile([P, FT], DT.float32, tag="bq", name="bq")
            bq = bq_full[:, :width]
            nc.scalar.activation(
                out=bq[:],
                in_=expl[:],
                func=mybir.ActivationFunctionType.Ln,
                bias=neg1[:, 0:1],
                accum_out=stats[:, COL_B + k : COL_B + k + 1],
            )
            nc.vector.scalar_tensor_tensor(
                out=scr[:, :width],
                in0=t_sl,
                scalar=0.5,
                in1=bq[:],
                op0=mybir.AluOpType.is_gt,
                op1=mybir.AluOpType.mult,
                accum_out=stats[:, COL_POSB + k : COL_POSB + k + 1],
            )

        for t in range(NTILES):
            fsl = slice(t * FT, (t + 1) * FT)
            last = t == NTILES - 1
            target_chunk_prep(t)
            # PSUM acc: full tiles [A(1024) | L_sel(1024)];
            # last tile [A_h0(512) | Lsel_h0(512) | A_h1(512) | Lsel_h1(512)]
            acc_ps = psum_pool.tile([P, 2 * FT], DT.float32, tag="acc")

            for c in range(C):
                p_t = p_pool.tile([P, FT], DT.float32, tag="p")
                # p bufs=8 aligns slot reuse with the global DMA->DMAHW-proc
                # round-robin (8 procs), so the WAW on the old writer is
                # same-proc FIFO order and Tile emits no cross-queue wait
                nc.sync.dma_start(out=p_t[:], in_=pred_r[c, :, fsl])
                if not last:
                    class_pass(p_t[:], t_bf[:, fsl], FT, acc_ps, 0, c)
                else:
                    for h in range(2):
                        gsl = slice(t * FT + h * HALF, t * FT + (h + 1) * HALF)
                        class_pass(
                            p_t[:, h * HALF : (h + 1) * HALF],
                            t_bf[:, gsl],
                            HALF,
                            acc_ps,
                            h * 2 * HALF,
                            c,
                        )

            if not last:
                emit_tail(t_bf[:, fsl], acc_ps[:, :FT], acc_ps[:, FT:], t, FT)
            else:
                for h in range(2):
                    gsl = slice(t * FT + h * HALF, t * FT + (h + 1) * HALF)
                    emit_tail(
                        t_bf[:, gsl],
                        acc_ps[:, h * 2 * HALF : h * 2 * HALF + HALF],
                        acc_ps[:, h * 2 * HALF + HALF : (h + 1) * 2 * HALF],
                        NTILES - 1 + h,
                        HALF,
                    )

        assert ucol[0] == COL_UALL + N_UALL

        # ship the raw stats tile; the host folds partitions/columns in f64
        nc.sync.dma_start(out=out[:], in_=stats[:])

    if not nc.is_finalized():
        nc.finalize()

    return nc


def combine_stats(stats_list) -> np.float32:
    """Host-side fold of the per-core [P, STAT_COLS] stats tiles (f64)."""
    u_all = b_sum = pos_b = pos_a = pos = np.float64(0.0)
    for st in stats_list:
        st = np.asarray(st, dtype=np.float64).reshape(P, STAT_COLS)
        u_all += st[:, COL_UALL : COL_UALL + N_UALL].sum()
        b_sum += st[:, COL_B : COL_B + N_TAILS].sum()
        pos_b += st[:, COL_POSB : COL_POSB + N_TAILS].sum()
        pos_a += st[:, COL_POSA : COL_POSA + N_TAILS].sum()
        pos += st[:, COL_CNT : COL_CNT + NTILES].sum()
    tot = np.float64(len(stats_list) * PIX)
    s_all = -(b_sum + u_all)
    s_pos = -(pos_b + pos_a)
    neg = tot - pos
    s_neg = s_all - s_pos
    loss = ((neg / tot) * s_pos + (pos / tot) * s_neg) / (tot * C)
    return np.float32(loss)


def make_in_maps(predict: np.ndarray, target: np.ndarray):
    import ml_dtypes

    predict = np.ascontiguousarray(predict, dtype=np.float32)
    target = np.ascontiguousarray(target, dtype=np.int32)
    idn = np.eye(P, dtype=np.float32).astype(ml_dtypes.bfloat16)
    return [
        {
            "predict": predict[k].reshape(C, PIX),
            "target": target[k].reshape(P, FCOLS),
            "idn": idn,
        }
        for k in range(N_CORES)
    ]


_NC_CACHE = None


def kernel(predict: np.ndarray, target: np.ndarray) -> np.ndarray:
    global _NC_CACHE
    if _NC_CACHE is None:
        _NC_CACHE = build_kernel()
    nc = _NC_CACHE

    in_maps = make_in_maps(predict, target)
    res = run_bass_kernel_spmd(nc, in_maps, list(range(N_CORES)))
    return combine_stats([res.results[k]["out"] for k in range(N_CORES)])


# revision 8
# speedup vs baseline: 1.0963x; 1.0963x over previous
"""Weighted 2D cross-entropy (BCE-over-classes) loss on 8 Trainium2 cores.

Math (matches the reference):
  t in [0,19); pos = t>0, neg = t==0 (all pixels are pos or neg; mask == 1)
  S(i) = sum_c bce(i,c) = -[ B(i) + A(i) ]
     A(i) = sum_c log(1-p_c(i))
     B(i) = log(p_t(i)) - log(1-p_t(i))
  loss = ( (NEG/TOT)*S_pos_sum + (POS/TOT)*S_neg_sum ) / (TOT*C)

Per-core (core k <- batch element k, pure data parallel):
  - per class-tile: ACT pass L_c = Ln(1-p_c) in bf16 with accum_out
    giving the U_all = sum A column sums for free
  - DVE: eq mask (T==c) and masked_c = eq*L_c
  - PE identity-matmuls accumulate A = sum_c L_c and L_sel = sum_c masked_c
    into PSUM (f32) -- the gather of log(1-p) at the target class.
  - per-tile tail: B = Ln(exp(-L_sel) - 1) = log(p_t) - log(1-p_t) in two
    ACT ops (accum_out on the Ln gives sum B free); two DVE STTs give the
    pos-masked sums of A and B.
  - the last pixel tile is processed in two 512-col halves so the final
    dependency chain after the last DMA byte is short.
  - the whole [128, STAT_COLS] stats tile is DMA'd out; the host does all
    folding (partition+column sums and the cross-core "all-reduce") in f64.
"""

from contextlib import ExitStack

import numpy as np

import concourse.bass as bass
import concourse.mybir as mybir
import concourse.tile as tile
from concourse import bacc
from concourse.bass_utils import run_bass_kernel_spmd

# problem shape (hardcoded per harness contract)
N, C, H, W = 8, 19, 512, 1024
PIX = H * W          # 524288 pixels per core
P = 128              # partitions
FCOLS = PIX // P     # 4096 free columns when pixels laid out [128, 4096]
FT = 1024            # pixel-tile free width
NTILES = FCOLS // FT # 4 pixel tiles per core
HALF = FT // 2       # last tile processed as two 512-col halves
N_CORES = 8

DT = mybir.dt

# stats buffer column layout (all f32, each column written exactly once)
N_TAILS = (NTILES - 1) + 2              # 3 full tails + 2 half tails = 5
COL_UALL = 0                            # sum A per tail
N_UALL = N_TAILS
COL_B = COL_UALL + N_TAILS              # sum B per tail
COL_POSB = COL_B + N_TAILS              # sum pos*B per tail
COL_POSA = COL_POSB + N_TAILS           # sum pos*A per tail
COL_CNT = COL_POSA + N_TAILS            # pos count per target chunk
STAT_COLS = COL_CNT + NTILES            # 24
NSTAT = STAT_COLS  # legacy alias


def build_kernel() -> bass.Bass:
    # Bacc (not raw Bass): its compile() pipeline runs
    # generate_event_semaphores, which splits multi-sem waits to satisfy the
    # 1-wait-per-instruction TRN2 sync structs -- raw Bass modules with
    # Tile-emitted multi-waits fail walrus codegen.
    nc = bacc.Bacc("TRN2")

    predict = nc.declare_dram_parameter("predict", [C, PIX], DT.float32, isOutput=False)
    target = nc.declare_dram_parameter("target", [P, FCOLS], DT.int32, isOutput=False)
    idn = nc.declare_dram_parameter("idn", [P, P], DT.bfloat16, isOutput=False)
    out = nc.declare_dram_parameter("out", [P, STAT_COLS], DT.float32, isOutput=True)

    pred_r = predict.rearrange("c (p f) -> c p f", p=P)  # [19, 128, 4096]

    with tile.TileContext(nc) as tc, ExitStack() as ctx:
        const = ctx.enter_context(tc.tile_pool(name="const", bufs=1))
        p_pool = ctx.enter_context(tc.tile_pool(name="p", bufs=8))
        lm_pool = ctx.enter_context(tc.tile_pool(name="lm", bufs=21))
        pix_pool = ctx.enter_context(tc.tile_pool(name="pix", bufs=2))
        scr_pool = ctx.enter_context(tc.tile_pool(name="scr", bufs=2))
        eq_pool = ctx.enter_context(tc.tile_pool(name="eq", bufs=4))
        psum_pool = ctx.enter_context(tc.tile_pool(name="ps", bufs=2, space="PSUM"))

        # constants + target go through the gpsimd queue so the sync queue's
        # first instruction is the first predict load (DMA starts at t~0)
        idn_sb = const.tile([P, P], DT.bfloat16, tag="idn")
        nc.gpsimd.dma_start(out=idn_sb[:], in_=idn[:])

        t_i32 = const.tile([P, FCOLS], DT.int32, tag="ti")
        for t in range(NTILES):
            sl = slice(t * FT, (t + 1) * FT)
            nc.gpsimd.dma_start(out=t_i32[:, sl], in_=target[:, sl])

        t_bf = const.tile([P, FCOLS], DT.bfloat16, tag="tb")
        stats = const.tile([P, STAT_COLS], DT.float32, tag="stats")
        cnt_scr = const.tile([P, FT], DT.bfloat16, tag="cntscr")
        neg1 = const.tile([P, 1], DT.float32, tag="neg1")
        nc.gpsimd.memset(neg1[:], -1.0)

        def target_chunk_prep(t):
            # int->bf16 conversion + pos count for chunk t; emitted just
            # before tile t's class loop so the DVE queue head never blocks
            # on a not-yet-arrived target chunk
            sl = slice(t * FT, (t + 1) * FT)
            nc.vector.tensor_copy(out=t_bf[:, sl], in_=t_i32[:, sl])
            nc.vector.tensor_scalar(
                out=cnt_scr[:],
                in0=t_bf[:, sl],
                scalar1=0.5,
                scalar2=None,
                op0=mybir.AluOpType.is_gt,
                op1=mybir.AluOpType.add,
                accum_out=stats[:, COL_CNT + t : COL_CNT + t + 1],
            )

        def class_pass(p_src, t_sl, lm_w, acc_ps, acc_off, c):
            """One class over lm_w pixel columns: Ln, eq, mask, PE accumulate."""
            lm_full = lm_pool.tile([P, 2 * FT], DT.bfloat16, tag="lm", name="lm")
            lm = lm_full[:, : 2 * lm_w]
            nc.scalar.activation(
                out=lm[:, :lm_w],
                in_=p_src,
                func=mybir.ActivationFunctionType.Ln,
                bias=1.0,
                scale=-1.0,
            )
            eq_full = eq_pool.tile([P, FT], DT.bfloat16, tag="eq", name="eq")
            eq = eq_full[:, :lm_w]
            nc.vector.tensor_scalar(
                out=eq[:],
                in0=t_sl,
                scalar1=float(c),
                scalar2=None,
                op0=mybir.AluOpType.is_equal,
            )
            nc.vector.tensor_mul(out=lm[:, lm_w:], in0=eq[:], in1=lm[:, :lm_w])
            for s in range(2 * lm_w // 512):
                ssl = slice(s * 512, (s + 1) * 512)
                asl = slice(acc_off + s * 512, acc_off + (s + 1) * 512)
                nc.tensor.matmul(
                    acc_ps[:, asl],
                    lhsT=idn_sb[:],
                    rhs=lm[:, ssl],
                    start=(c == 0),
                    stop=(c == C - 1),
                )

        def emit_tail(t_sl, a_ps, lsel_ps, k, width):
            """Tail over one pixel range: pos*A, B = Ln(exp(-lsel)-1), pos*B."""
            nc.vector.tensor_reduce(
                out=stats[:, COL_UALL + k : COL_UALL + k + 1],
                in_=a_ps,
                axis=mybir.AxisListType.X,
                op=mybir.AluOpType.add,
            )
            scr = scr_pool.tile([P, FT], DT.float32, tag="scr")
            nc.vector.scalar_tensor_tensor(
                out=scr[:, :width],
                in0=t_sl,
                scalar=0.5,
                in1=a_ps,
                op0=mybir.AluOpType.is_gt,
                op1=mybir.AluOpType.mult,
                accum_out=stats[:, COL_POSA + k : COL_POSA + k + 1],
            )
            expl_full = pix_pool.tile([P, FT], DT.float32, tag="expl", name="expl")
            expl = expl_full[:, :width]
            nc.scalar.activation(
                out=expl[:],
                in_=lsel_ps,
                func=mybir.ActivationFunctionType.Exp,
                scale=-1.0,
            )
            bq_full = pix_pool.tile([P, FT], DT.float32, tag="bq", name="bq")
            bq = bq_full[:, :width]
            nc.scalar.activation(
                out=bq[:],
                in_=expl[:],
                func=mybir.ActivationFunctionType.Ln,
                bias=neg1[:, 0:1],
            )
            nc.vector.tensor_reduce(
                out=stats[:, COL_B + k : COL_B + k + 1],
                in_=bq[:],
                axis=mybir.AxisListType.X,
                op=mybir.AluOpType.add,
            )
            nc.vector.scalar_tensor_tensor(
                out=scr[:, :width],
                in0=t_sl,
                scalar=0.5,
                in1=bq[:],
                op0=mybir.AluOpType.is_gt,
                op1=mybir.AluOpType.mult,
                accum_out=stats[:, COL_POSB + k : COL_POSB + k + 1],
            )

        for t in range(NTILES):
            fsl = slice(t * FT, (t + 1) * FT)
            last = t == NTILES - 1
            target_chunk_prep(t)
            # PSUM acc: full tiles [A(1024) | L_sel(1024)];
            # last tile [A_h0(512) | Lsel_h0(512) | A_h1(512) | Lsel_h1(512)]
            acc_ps = psum_pool.tile([P, 2 * FT], DT.float32, tag="acc")

            for c in range(C):
                p_t = p_pool.tile([P, FT], DT.float32, tag="p")
                # p bufs=8 aligns slot reuse with the global DMA->DMAHW-proc
                # round-robin (8 procs), so the WAW on the old writer is
                # same-proc FIFO order and Tile emits no cross-queue wait
                nc.sync.dma_start(out=p_t[:], in_=pred_r[c, :, fsl])
                if not last:
                    class_pass(p_t[:], t_bf[:, fsl], FT, acc_ps, 0, c)
                else:
                    for h in range(2):
                        gsl = slice(t * FT + h * HALF, t * FT + (h + 1) * HALF)
                        class_pass(
                            p_t[:, h * HALF : (h + 1) * HALF],
                            t_bf[:, gsl],
                            HALF,
                            acc_ps,
                            h * 2 * HALF,
                            c,
                        )

            if not last:
                emit_tail(t_bf[:, fsl], acc_ps[:, :FT], acc_ps[:, FT:], t, FT)
            else:
                for h in range(2):
                    gsl = slice(t * FT + h * HALF, t * FT + (h + 1) * HALF)
                    emit_tail(
                        t_bf[:, gsl],
                        acc_ps[:, h * 2 * HALF : h * 2 * HALF + HALF],
                        acc_ps[:, h * 2 * HALF + HALF : (h + 1) * 2 * HALF],
                        NTILES - 1 + h,
                        HALF,
                    )

        # ship the raw stats tile; the host folds partitions/columns in f64
        nc.sync.dma_start(out=out[:], in_=stats[:])

    if not nc.is_finalized():
        nc.finalize()

    return nc


def combine_stats(stats_list) -> np.float32:
    """Host-side fold of the per-core [P, STAT_COLS] stats tiles (f64)."""
    u_all = b_sum = pos_b = pos_a = pos = np.float64(0.0)
    for st in stats_list:
        st = np.asarray(st, dtype=np.float64).reshape(P, STAT_COLS)
        u_all += st[:, COL_UALL : COL_UALL + N_UALL].sum()
        b_sum += st[:, COL_B : COL_B + N_TAILS].sum()
        pos_b += st[:, COL_POSB : COL_POSB + N_TAILS].sum()
        pos_a += st[:, COL_POSA : COL_POSA + N_TAILS].sum()
        pos += st[:, COL_CNT : COL_CNT + NTILES].sum()
    tot = np.float64(len(stats_list) * PIX)
    s_all = -(b_sum + u_all)
    s_pos = -(pos_b + pos_a)
    neg = tot - pos
    s_neg = s_all - s_pos
    loss = ((neg / tot) * s_pos + (pos / tot) * s_neg) / (tot * C)
    return np.float32(loss)


def make_in_maps(predict: np.ndarray, target: np.ndarray):
    import ml_dtypes

    predict = np.ascontiguousarray(predict, dtype=np.float32)
    target = np.ascontiguousarray(target, dtype=np.int32)
    idn = np.eye(P, dtype=np.float32).astype(ml_dtypes.bfloat16)
    return [
        {
            "predict": predict[k].reshape(C, PIX),
            "target": target[k].reshape(P, FCOLS),
            "idn": idn,
        }
        for k in range(N_CORES)
    ]


_NC_CACHE = None


def kernel(predict: np.ndarray, target: np.ndarray) -> np.ndarray:
    global _NC_CACHE
    if _NC_CACHE is None:
        _NC_CACHE = build_kernel()
    nc = _NC_CACHE

    in_maps = make_in_maps(predict, target)
    res = run_bass_kernel_spmd(nc, in_maps, list(range(N_CORES)))
    return combine_stats([res.results[k]["out"] for k in range(N_CORES)])
